# revision 39
# baseline (speedup 1.0000x reference)
"""Causal self-attention (B=2, S=2048, H=1024, 16 heads) on 8 trn2 NeuronCores.

Sharding: core c handles batch b = c // 4 and head-group g = c % 4
(4 heads x d=64 = 256 output columns). Fully parallel, no collectives.

v3 pipeline (per core, all matmuls f32r = TF32-class):
  - x -> xT via f32r PE transposes (4 per PSUM bank, one [128,512] DVE evac)
  - QT/KT = W^T xT + b (d on partitions), V natural with [1 | V_h] interleave
  - scores^T per head PAIR packed into the PE array via tile_position
    (d=64 contraction fills half the rows); one exp over [128,1024] PSUM
  - causal diag masking: one half on DVE (0/1 mask mul), one on GPSIMD
  - PV: [1|V_h]^T @ expT accumulated in PSUM -> [65, 512]; row 0 = softmax
    denominator; tail = reciprocal + SBUF DMA-broadcast + one DVE multiply;
    output stored TRANSPOSED [256, 2048] per core, host transposes back
  - emission: software-pipelined score stream (2 ahead of PV) with the
    remaining projection/V/transpose work woven in as fine-grained fillers
    so ACT's exp pipeline saturates from ~10us onward
"""

from collections import deque

import numpy as np

import concourse.bacc as bacc
import concourse.mybir as mybir
from concourse.tile import TileContext
from concourse.bass_utils import run_bass_kernel_spmd
from concourse.masks import make_identity

B, S, H, NH, D = 2, 2048, 1024, 16, 64
P = 128
NCORES = 8
NHL = NH // 4            # 4 heads per core
HGD = NHL * D            # 256 output cols per core
HC = H // P              # 8 contraction chunks
SC = S // P              # 16 sequence chunks of 128
QC = S // 512            # 4 query chunks of 512
KC = S // P              # 16 key chunks of 128
DC = HGD // P            # 2 partition chunks of QT/KT

fp32 = mybir.dt.float32
f32r = mybir.dt.float32r
AF = mybir.ActivationFunctionType
ALU = mybir.AluOpType

_CACHE = {}
LAST_RESULTS = None


def _emit(nc):
    x = nc.declare_dram_parameter("x", [S, H], f32r, isOutput=False)
    wq = nc.declare_dram_parameter("wq", [H, HGD], f32r, isOutput=False)
    wk = nc.declare_dram_parameter("wk", [H, HGD], f32r, isOutput=False)
    wv = nc.declare_dram_parameter("wv", [H, HGD], f32r, isOutput=False)
    bq = nc.declare_dram_parameter("bq", [HGD], fp32, isOutput=False)
    bk = nc.declare_dram_parameter("bk", [HGD], fp32, isOutput=False)
    bv = nc.declare_dram_parameter("bv", [HGD], fp32, isOutput=False)
    mask = nc.declare_dram_parameter("mask", [S], fp32, isOutput=False)
    # transposed output: host does the final [HGD, S] -> [S, HGD] transpose
    out = nc.declare_dram_parameter("out", [HGD, S], fp32, isOutput=True)

    scale = float(1.0 / np.sqrt(np.float32(D)))

    with TileContext(nc) as tc:
        with tc.tile_pool(name="const", bufs=1) as const, \
             tc.tile_pool(name="big", bufs=1) as big:

            # ---- big tiles ----
            xT = big.tile([P, HC, S], f32r, tag="xT")
            QT = big.tile([P, DC, S], f32r, tag="QT")
            KT = big.tile([P, DC, S], f32r, tag="KT")
            VS, VOFF = 128, 64  # per-head [1 | zeros*63 | V] stationary layout
            Vt = big.tile([P, SC, NHL * VS], f32r, tag="Vt")
            Vt4 = Vt.rearrange("p sc (h c) -> p sc h c", c=VS)

            with tc.tile_pool(name="xin", bufs=4) as xin, \
                 tc.tile_pool(name="et", bufs=4) as etp, \
                 tc.tile_pool(name="rcp", bufs=2) as rcpp, \
                 tc.tile_pool(name="cnorm", bufs=2) as cnp, \
                 tc.tile_pool(name="psW", bufs=2, space="PSUM") as psW, \
                 tc.tile_pool(name="psE", bufs=1, space="PSUM") as psE:

                # ---------- early loads: x first (PE depends on it) ----------
                def emit_xload(sc):
                    xt = xin.tile([P, H], f32r, tag="xt", name="xt")
                    nc.sync.dma_start(xt[:], x[sc * P:(sc + 1) * P, :])
                    return xt

                xt0 = xin.tile([P, H], f32r, tag="xt", name="xt")
                nc.sync.dma_start(xt0[:, 0:512], x[0:P, 0:512])
                nc.sync.dma_start(xt0[:, 512:H], x[0:P, 512:H])
                early_xt = [xt0] + [emit_xload(sc) for sc in range(1, 4)]

                ident_f = const.tile([P, P], fp32, tag="identf")
                make_identity(nc, ident_f)
                ident_r = const.tile([P, P], f32r, tag="identr")
                nc.vector.tensor_copy(ident_r[:], ident_f[:])
                # 0/1 causal masks: cm[:, j, f] = (f - p >= -j*128)
                cm = const.tile([P, 4, 512], mybir.dt.bfloat16, tag="cmask")
                nc.gpsimd.memset(cm[:], 1.0)
                for j in range(4):
                    nc.gpsimd.affine_select(
                        out=cm[:, j, :], in_=cm[:, j, :],
                        compare_op=ALU.is_ge, fill=0.0,
                        base=-j * P, pattern=[[1, 512]], channel_multiplier=-1)

                # weights / biases / mask (needed ~15us in)
                wq_t = big.tile([P, HC, HGD], f32r, tag="wq")
                wk_t = big.tile([P, HC, HGD], f32r, tag="wk")
                wv_t = big.tile([P, HC, HGD], f32r, tag="wv")
                nc.sync.dma_start(wq_t[:], wq[:].rearrange("(hc p) n -> p hc n", p=P))
                nc.sync.dma_start(wk_t[:], wk[:].rearrange("(hc p) n -> p hc n", p=P))
                nc.sync.dma_start(wv_t[:], wv[:].rearrange("(hc p) n -> p hc n", p=P))
                bq_t = const.tile([P, DC], fp32, tag="bq")
                bk_t = const.tile([P, DC], fp32, tag="bk")
                nc.sync.dma_start(bq_t[:], bq[:].rearrange("(dc p) -> p dc", p=P))
                nc.sync.dma_start(bk_t[:], bk[:].rearrange("(dc p) -> p dc", p=P))
                bv_b = const.tile([P, HGD], fp32, tag="bv")
                nc.gpsimd.dma_start(bv_b[:], bv[None, :].to_broadcast([P, HGD]))
                bv4 = bv_b.rearrange("p (h c) -> p h c", c=D)
                mask_t = const.tile([P, KC], fp32, tag="mask")
                nc.sync.dma_start(mask_t[:], mask[:].rearrange("(kc p) -> p kc", p=P))

                zero_c = const.tile([P, 1], fp32, tag="zero")
                nc.vector.memset(zero_c[:], 0.0)
                ones_c = const.tile([P, 1], fp32, tag="ones")
                nc.vector.memset(ones_c[:], 1.0)

                def emit_xtr(xt, sc, hg):  # 4 transposes + 1 evac
                    tp = psW.tile([P, 512], f32r, tag="pp", name="tp")
                    for j in range(4):
                        hc = hg * 4 + j
                        nc.tensor.matmul(
                            tp[:, j * P:(j + 1) * P],
                            xt[:, hc * P:(hc + 1) * P], ident_r[:],
                            is_transpose=True, start=(j == 0), stop=(j == 3),
                            skip_group_check=True)
                    nc.vector.tensor_copy(
                        xT[:, hg * 4:(hg + 1) * 4, sc * P:(sc + 1) * P],
                        tp.rearrange("p (j c) -> p j c", c=P))

                def proj_closures(W, bias_t, OUT, dc, sq):
                    pp = [None]

                    def mk(hc):
                        def go():
                            if hc == 0:
                                pp[0] = psW.tile([P, 512], fp32, tag="pp", name="pp")
                            nc.tensor.matmul(
                                pp[0][:], W[:, hc, dc * P:(dc + 1) * P],
                                xT[:, hc, sq * 512:(sq + 1) * 512],
                                start=(hc == 0), stop=(hc == HC - 1))
                            if hc == HC - 1:
                                nc.vector.tensor_scalar_add(
                                    OUT[:, dc, sq * 512:(sq + 1) * 512],
                                    pp[0][:], bias_t[:, dc:dc + 1])
                        return go
                    return [mk(hc) for hc in range(HC)]

                def v_closures(scp):  # V for sc pair -> one [128,512] bank
                    pp = [None]

                    def mk(half, hc):
                        def go():
                            if half == 0 and hc == 0:
                                pp[0] = psW.tile([P, 512], fp32, tag="pp", name="pp")
                            sc = scp * 2 + half
                            nc.tensor.matmul(
                                pp[0][:, half * HGD:(half + 1) * HGD],
                                xT[:, hc, sc * P:(sc + 1) * P], wv_t[:, hc, :],
                                start=(half == 0 and hc == 0),
                                stop=(hc == HC - 1), skip_group_check=True)
                            if half == 1 and hc == HC - 1:
                                nc.vector.tensor_tensor(
                                    Vt4[:, scp * 2:scp * 2 + 2, :, VOFF:VOFF + D],
                                    pp[0].rearrange("p (s h c) -> p s h c",
                                                    s=2, c=D),
                                    bv4[:, None, :, :].to_broadcast(
                                        [P, 2, NHL, D]),
                                    ALU.add)
                        return go
                    return [mk(h, hc) for h in range(2) for hc in range(HC)]

                # ---------- filler queue with dependency markers ----------
                fillers = deque()
                markers = {}
                done = [0]

                def pull(n):
                    for _ in range(n):
                        if not fillers:
                            return
                        fillers.popleft()()
                        done[0] += 1

                def drain_to(marker):
                    tgt = markers.get(marker, 0)
                    while done[0] < tgt:
                        fillers.popleft()()
                        done[0] += 1

                def add_fillers(closures):
                    fillers.extend(closures)

                def set_marker(name):
                    markers[name] = done[0] + len(fillers)

                # ---------- attention ----------
                def sc_exp(pr, qc, kc):
                    # diagonal tiles (j >= 0): columns f < j*128 are fully
                    # masked -> skip them in scores, exp, mask and PV
                    q0 = qc * 512
                    j = kc - qc * 4
                    off = max(0, j) * P
                    w = 512 - off
                    QTa, QTb = QT[0:D, pr, :], QT[D:P, pr, :]
                    KTa, KTb = KT[0:D, pr, :], KT[D:P, pr, :]
                    sps = psE.tile([P, 1024], fp32, tag="sps", bufs=2, name="sps")
                    nc.tensor.matmul(
                        sps[:, off:512], KTa[:, kc * P:(kc + 1) * P],
                        QTa[:, q0 + off:q0 + 512], start=True, stop=True,
                        tile_position=(0, 0))
                    nc.tensor.matmul(
                        sps[:, 512 + off:1024], KTb[:, kc * P:(kc + 1) * P],
                        QTb[:, q0 + off:q0 + 512], start=True, stop=True,
                        tile_position=(64, 0))
                    et = etp.tile([P, 1024], f32r, tag="et", name="et")
                    sps2 = sps.rearrange("p (h f) -> p h f", h=2)
                    et2 = et.rearrange("p (h f) -> p h f", h=2)
                    nc.scalar.activation(et2[:, :, off:], sps2[:, :, off:],
                                         AF.Exp, scale=scale,
                                         bias=mask_t[:, kc:kc + 1])
                    if j >= 0:  # zero the partial rows of the triangle
                        nc.vector.tensor_mul(
                            et2[:, :, off:], et2[:, :, off:],
                            cm[:, None, j, off:].to_broadcast([P, 2, w]))
                    return et, off

                def pv(pr, kc, nkc, etoff, ctxa, ctxb):
                    et, off = etoff
                    ha, hb = 2 * pr, 2 * pr + 1
                    nc.tensor.matmul(
                        ctxa[:, off:], Vt[:, kc, ha * VS:(ha + 1) * VS],
                        et[:, off:512], start=(kc == 0), stop=(kc == nkc - 1))
                    nc.tensor.matmul(
                        ctxb[:, off:], Vt[:, kc, hb * VS:(hb + 1) * VS],
                        et[:, 512 + off:1024], start=(kc == 0),
                        stop=(kc == nkc - 1))

                def tail(h, qc, ctx):
                    q0 = qc * 512
                    rcp = rcpp.tile([1, 512], fp32, tag="rcp", name="rcp")
                    nc.vector.reciprocal(rcp[0:1, :], ctx[0:1, :])
                    rb = rcpp.tile([VOFF + D, 512], fp32, tag="rb", name="rb")
                    nc.gpsimd.partition_broadcast(rb[:], rcp[0:1, :])
                    ctxn = cnp.tile([VOFF + D, 512], fp32, tag="cn", name="cn")
                    nc.vector.tensor_mul(ctxn[VOFF:, :], ctx[VOFF:VOFF + D, :],
                                         rb[VOFF:, :])
                    nc.sync.dma_start(
                        out[h * D:(h + 1) * D, q0:q0 + 512], ctxn[VOFF:, :])

                # ---------- schedule ----------
                # prologue block 0: x(sc0..3) -> xT, QT/KT dc0 sq0, V scp0..1
                for sc in range(4):
                    emit_xtr(early_xt[sc], sc, 0)
                    emit_xtr(early_xt[sc], sc, 1)
                for cl in proj_closures(wq_t, bq_t, QT, 0, 0):
                    cl()
                for cl in proj_closures(wk_t, bk_t, KT, 0, 0):
                    cl()
                for cl in v_closures(0) + v_closures(1):
                    cl()
                # Vt fixed columns (after block-0 evacs so they don't block
                # the first xT evacuations in DVE's queue)
                nc.vector.tensor_copy(
                    Vt4[:, :, :, 1:VOFF],
                    zero_c[:, 0:1, None, None].to_broadcast(
                        [P, SC, NHL, VOFF - 1]))
                nc.vector.tensor_copy(
                    Vt4[:, :, :, 0],
                    ones_c[:, 0:1, None].to_broadcast([P, SC, NHL]))

                # filler blocks 1..3 + C dc1 (+ dc0 later-sq), with markers
                for g in range(1, 4):
                    def blk(g=g):
                        loads, trs = [], []
                        boxes = {}
                        for sc in range(4 * g, 4 * g + 4):
                            boxes[sc] = []

                            def load(sc=sc):
                                boxes[sc].append(emit_xload(sc))

                            loads.append(load)
                            for hg in range(2):
                                def tr(sc=sc, hg=hg):
                                    emit_xtr(boxes[sc][0], sc, hg)
                                trs.append(tr)
                        # 2-deep DMA lookahead: L L t t L t t L t t t t
                        out_cl = [loads[0], loads[1], trs[0], trs[1],
                                  loads[2], trs[2], trs[3],
                                  loads[3], trs[4], trs[5], trs[6], trs[7]]
                        out_cl += proj_closures(wq_t, bq_t, QT, 0, g)
                        out_cl += proj_closures(wk_t, bk_t, KT, 0, g)
                        out_cl += v_closures(2 * g)
                        out_cl += v_closures(2 * g + 1)
                        return out_cl
                    add_fillers(blk())
                    set_marker(("blk", g))
                for sq in range(QC):
                    add_fillers(proj_closures(wk_t, bk_t, KT, 1, sq))
                for sq in (3, 2, 1, 0):
                    add_fillers(proj_closures(wq_t, bq_t, QT, 1, sq))
                    set_marker(("cdc1", sq))

                for pr in range(2):
                    qcs = list(range(QC)) if pr == 0 else list(range(QC))[::-1]
                    flat = [(qc, kc) for qc in qcs
                            for kc in range(4 * (qc + 1))]
                    ctxs = {}
                    ets = {}

                    def ensure(qc):
                        if pr == 0:
                            if qc > 0:
                                drain_to(("blk", qc))
                        else:
                            drain_to(("cdc1", qc))

                    def start_unit(qc):
                        ensure(qc)
                        ctxs[qc] = (
                            psE.tile([VOFF + D, 512], fp32, tag="ctx", bufs=2, name="ctx"),
                            psE.tile([VOFF + D, 512], fp32, tag="ctx", bufs=2, name="ctx"))

                    start_unit(flat[0][0])
                    for ahead in range(2):
                        qc, kc = flat[ahead]
                        ets[(qc, kc)] = sc_exp(pr, qc, kc)
                    for i, (qc, kc) in enumerate(flat):
                        nkc = 4 * (qc + 1)
                        pv(pr, kc, nkc, ets.pop((qc, kc)),
                           ctxs[qc][0], ctxs[qc][1])
                        pull(3)
                        if i + 2 < len(flat):
                            q2, k2 = flat[i + 2]
                            if k2 == 0:
                                start_unit(q2)
                            ets[(q2, k2)] = sc_exp(pr, q2, k2)
                        if kc == nkc - 1:
                            ca, cb = ctxs.pop(qc)
                            tail(2 * pr, qc, ca)
                            tail(2 * pr + 1, qc, cb)
                # drain any remaining fillers (shouldn't be many)
                while fillers:
                    pull(1)


def build():
    if "nc" not in _CACHE:
        nc = bacc.Bacc("TRN2", target_bir_lowering=False, debug=False,
                       num_devices=NCORES)
        _emit(nc)
        nc.compile()
        _CACHE["nc"] = nc
    return _CACHE["nc"]


def make_in_maps(hidden_states, attention_mask, Wq, bq, Wk, bk, Wv, bv):
    in_maps = []
    for c in range(NCORES):
        b, g = c // 4, c % 4
        sl = slice(g * HGD, (g + 1) * HGD)
        in_maps.append({
            "x": np.ascontiguousarray(hidden_states[b]),
            "wq": np.ascontiguousarray(Wq[:, sl]),
            "wk": np.ascontiguousarray(Wk[:, sl]),
            "wv": np.ascontiguousarray(Wv[:, sl]),
            "bq": np.ascontiguousarray(bq[sl]),
            "bk": np.ascontiguousarray(bk[sl]),
            "bv": np.ascontiguousarray(bv[sl]),
            "mask": np.ascontiguousarray(attention_mask[b, 0, 0, :]),
        })
    return in_maps


def kernel(hidden_states, attention_mask, Wq, bq, Wk, bk, Wv, bv, **run_kwargs):
    global LAST_RESULTS
    hidden_states = np.asarray(hidden_states, dtype=np.float32)
    attention_mask = np.asarray(attention_mask, dtype=np.float32)
    nc = build()
    in_maps = make_in_maps(
        hidden_states, attention_mask,
        np.asarray(Wq, np.float32), np.asarray(bq, np.float32),
        np.asarray(Wk, np.float32), np.asarray(bk, np.float32),
        np.asarray(Wv, np.float32), np.asarray(bv, np.float32))
    res = run_bass_kernel_spmd(nc, in_maps, core_ids=list(range(NCORES)),
                               **run_kwargs)
    LAST_RESULTS = res
    full = np.empty((B, S, H), dtype=np.float32)
    for c in range(NCORES):
        b, g = c // 4, c % 4
        full[b, :, g * HGD:(g + 1) * HGD] = res.results[c]["out"].T
    return full
